# revision 1
# baseline (speedup 1.0000x reference)
"""Trainium2 Bass kernel for a custom GRU cell.

    x_h   = concat([inputs, h_prev], -1)            # [B, D+U]
    z     = sigmoid(x_h @ Wz)                       # [B, U]
    r     = sigmoid(x_h @ Wr)                       # [B, U]
    h_hat = tanh(concat([inputs, r * h_prev]) @ Wh) # [B, U]
    out   = z * h_prev + (1 - z) * h_hat

Data-parallel over 8 NeuronCores: batch sharded, weights replicated.

Per-core (B_c = 2048 rows, processed as 4 blocks of 512):
  - all matmuls in f32r (fp32 HIGH-half mode: ~bf16 speed, ~2^-13 precision)
  - x_h is transposed on the PE (f32r transpose-mode, 128x128 blocks) into
    feature-major k-tiles xh[k] [128, 512-batch], staged through PSUM
    and copied to SBUF by ScalarE
  - gate z batch-major:  psum[b,u]  = xh[k][:,bslice].T @ Wz[k]
  - gate r TRANSPOSED:   psum[u,b]  = Wr[k][:,uslice].T @ xh[k]
    so r*h_prev is computed transposed (rT * hT, where hT = xh[4+u]) with
    no extra transposes, and feeds gate h as lhsT directly
  - gate h batch-major, tanh/sigmoid on ScalarE straight out of PSUM,
    combine on VectorE, DMA out
  - emission is software-pipelined: gate-h of block i after z/r of i+1
"""

import sys

for _p in ("/opt/trn_rl_repo", "/root/.axon_site/_ro/trn_rl_repo"):
    if _p not in sys.path:
        sys.path.append(_p)

import numpy as np

B, D, U = 16384, 512, 512
K = D + U
N_CORES = 8
BC = B // N_CORES          # rows per core (2048)
BB = 512                   # batch-block rows
NB = BC // BB              # blocks per core (4)
KC = K // 128              # contraction chunks (8)


def build_gru_tile_kernel(tc, d_in, d_hp, d_wz, d_wr, d_wh, d_out, nb=NB):
    """Emit the GRU cell body into TileContext `tc`."""
    import contextlib

    from concourse import mybir
    from concourse.masks import make_identity

    f32 = mybir.dt.float32
    f32r = mybir.dt.float32r
    nc = tc.nc
    Sig = mybir.ActivationFunctionType.Sigmoid
    Tanh = mybir.ActivationFunctionType.Tanh

    est = contextlib.ExitStack()
    sing = est.enter_context(tc.tile_pool(name="sing", bufs=1))
    wpool = est.enter_context(tc.tile_pool(name="w", bufs=1))
    io = est.enter_context(tc.tile_pool(name="io", bufs=8))
    hpool = est.enter_context(tc.tile_pool(name="hpool", bufs=8))
    xhp = est.enter_context(tc.tile_pool(name="xhp", bufs=16))
    rhp = est.enter_context(tc.tile_pool(name="rhp", bufs=6))
    actp = est.enter_context(tc.tile_pool(name="act", bufs=6))
    tmpp = est.enter_context(tc.tile_pool(name="tmp", bufs=4))
    # PSUM: 3 transpose staging banks + 5 gate banks = 8/8
    # PSUM: 3 transpose staging banks + 5 gate banks = 8/8
    pst = est.enter_context(tc.tile_pool(name="pst", bufs=3, space="PSUM"))
    psg = est.enter_context(tc.tile_pool(name="psg", bufs=5, space="PSUM"))

    ident0 = sing.tile([128, 128], f32)
    make_identity(nc, ident0)
    identr = sing.tile([128, 128], f32r)
    nc.scalar.copy(identr[:], ident0[:])

    # ---- DMA schedule ----
    # The DMA engines drain instructions roughly in issue order, so load
    # block 0/1 activations first (unblocks the PE transposes ~12us in),
    # then stream the weights per-chunk (each z/r/h matmul only waits on
    # its own chunk), interleaved with the remaining blocks.
    pre_x = {}
    def load_x(bb):
        xin, hps = [], []
        for j in range(4):
            r0 = bb * BB + 128 * j
            x_j = io.tile([128, 512], f32r, tag="xin", name=f"x_{bb}_{j}")
            nc.sync.dma_start(x_j[:], d_in[r0:r0 + 128, :].bitcast(f32r))
            xin.append(x_j)
            h_j = hpool.tile([128, 512], f32r, tag="hp", name=f"h_{bb}_{j}")
            nc.sync.dma_start(h_j[:], d_hp[r0:r0 + 128, :].bitcast(f32r))
            hps.append(h_j)
        pre_x[bb] = (xin, hps)

    w_sb = {}
    def load_w(name, dram):
        t = wpool.tile([128, KC, 512], f32r, tag=name, name=name)
        for k in range(KC):
            nc.sync.dma_start(t[:, k, :], dram[128 * k:128 * (k + 1), :].bitcast(f32r))
        w_sb[name] = t

    load_x(0)
    load_w("wz", d_wz)
    load_w("wr", d_wr)
    if nb > 1:
        load_x(1)
    load_w("wh", d_wh)
    for bb in range(2, nb):
        load_x(bb)

    state = [None] * nb

    def phase_zr(bb):
        xin, hps = pre_x[bb]

        # ---- PE-transpose into feature-major k-tiles xh[k] [128, 512b] ----
        xh = [None] * KC
        xh_tiles = []

        def transpose_group(k):
            ps1 = pst.tile([128, 512], f32r, tag="pst", name=f"pst_{bb}_{k}")
            src = xin if k < 4 else hps
            kk = k % 4
            for j in range(4):
                nc.tensor.transpose(ps1[:, 128 * j:128 * (j + 1)],
                                    src[j][:, 128 * kk:128 * (kk + 1)], identr[:])
            sb1 = xhp.tile([128, 512], f32r, tag="xh", name=f"xh_{bb}_{k}")
            nc.scalar.copy(sb1[:], ps1[:])
            xh_tiles.append(sb1)
            xh[k] = sb1[:]

        for k in range(KC):
            transpose_group(k)

        # gate z, batch-major: ps[b,u] += xh[k][:,j].T @ Wz[k]
        zs = []
        for j in range(4):
            ps = psg.tile([128, 512], f32, tag="psg", name=f"psz_{bb}_{j}")
            for k in range(KC):
                nc.tensor.matmul(ps[:], xh[k][:, 128 * j:128 * (j + 1)],
                                 w_sb["wz"][:, k, :],
                                 start=(k == 0), stop=(k == KC - 1))
            z_j = tmpp.tile([128, 512], f32, tag="tmp", name=f"z_{bb}_{j}")
            nc.scalar.activation(z_j[:], ps[:], Sig)
            # Precompute zc = 1 - z (ACT) and zh = z * h_prev (DVE) now, so
            # the post-tanh chain in phase_h is only two VectorE ops.
            zc_j = actp.tile([128, 512], f32, tag="zc", name=f"zc_{bb}_{j}")
            nc.scalar.activation(zc_j[:], z_j[:],
                                 mybir.ActivationFunctionType.Copy,
                                 bias=1.0, scale=-1.0)
            zh_j = actp.tile([128, 512], f32, tag="zh", name=f"zh_{bb}_{j}")
            nc.vector.tensor_mul(zh_j[:], z_j[:], hps[j][:].bitcast(f32))
            zs.append((zc_j, zh_j))

        # ---- gate r, transposed: ps[u,b] += Wr[k][:,u].T @ xh[k] ----
        rhT = []
        for u in range(4):
            ps = psg.tile([128, 512], f32, tag="psg", name=f"psr_{bb}_{u}")
            for k in range(KC):
                nc.tensor.matmul(ps[:], w_sb["wr"][:, k, 128 * u:128 * (u + 1)],
                                 xh[k], start=(k == 0), stop=(k == KC - 1))
            rT_u = actp.tile([128, 512], f32, tag="rT", name=f"rT_{bb}_{u}")
            nc.scalar.activation(rT_u[:], ps[:], Sig)
            # rhT[u] = rT[u] * h_prev.T[u]  (hT = xh[4+u]), f32r out
            rh_u = rhp.tile([128, 512], f32r, tag="rhT", name=f"rh_{bb}_{u}")
            nc.vector.tensor_mul(rh_u[:], rT_u[:], xh[4 + u].bitcast(f32))
            rhT.append(rh_u)

        state[bb] = (xh, xh_tiles, hps, zs, rhT)

    def phase_h(bb):
        xh, xh_tiles, hps, zs, rhT = state[bb]
        for j in range(4):
            ps = psg.tile([128, 512], f32, tag="psg", name=f"psh_{bb}_{j}")
            for k in range(KC):
                lhs = (xh[k][:, 128 * j:128 * (j + 1)] if k < 4
                       else rhT[k - 4][:, 128 * j:128 * (j + 1)])
                nc.tensor.matmul(ps[:], lhs, w_sb["wh"][:, k, :],
                                 start=(k == 0), stop=(k == KC - 1))
            hh = actp.tile([128, 512], f32, tag="hh", name=f"hh_{bb}_{j}")
            nc.scalar.activation(hh[:], ps[:], Tanh)

            # out = (1 - z) * hh + z * hp, with both z-terms precomputed
            zc_j, zh_j = zs[j]
            t2 = tmpp.tile([128, 512], f32, tag="tmp", name=f"t2_{bb}_{j}")
            nc.vector.tensor_mul(t2[:], zc_j[:], hh[:])
            out = tmpp.tile([128, 512], f32, tag="out", name=f"o_{bb}_{j}")
            nc.vector.tensor_add(out[:], t2[:], zh_j[:])
            r0 = bb * BB + 128 * j
            nc.sync.dma_start(d_out[r0:r0 + 128, :], out[:])
        state[bb] = None

    phase_zr(0)
    for bb in range(1, nb):
        phase_zr(bb)
        phase_h(bb - 1)
    phase_h(nb - 1)

    est.close()


_NC_CACHE = {}


def _build(nb=NB):
    if nb in _NC_CACHE:
        return _NC_CACHE[nb]
    import concourse.tile as tile
    from concourse import bacc, mybir

    f32 = mybir.dt.float32
    nc = bacc.Bacc("TRN2", target_bir_lowering=False, debug=False)
    d_in = nc.dram_tensor("inputs", [nb * BB, D], f32, kind="ExternalInput").ap()
    d_hp = nc.dram_tensor("h_prev", [nb * BB, U], f32, kind="ExternalInput").ap()
    d_wz = nc.dram_tensor("Wz", [K, U], f32, kind="ExternalInput").ap()
    d_wr = nc.dram_tensor("Wr", [K, U], f32, kind="ExternalInput").ap()
    d_wh = nc.dram_tensor("Wh", [K, U], f32, kind="ExternalInput").ap()
    d_out = nc.dram_tensor("out", [nb * BB, U], f32, kind="ExternalOutput").ap()

    with tile.TileContext(nc) as tc:
        build_gru_tile_kernel(tc, d_in, d_hp, d_wz, d_wr, d_wh, d_out, nb=nb)
    nc.compile()
    _NC_CACHE[nb] = nc
    return nc


def run_sharded(inputs, h_prev, Wz, Wr, Wh, trace=False):
    from concourse.bass_utils import run_bass_kernel_spmd

    nc = _build()
    inputs = np.ascontiguousarray(np.asarray(inputs, dtype=np.float32))
    h_prev = np.ascontiguousarray(np.asarray(h_prev, dtype=np.float32))
    Wz = np.ascontiguousarray(np.asarray(Wz, dtype=np.float32))
    Wr = np.ascontiguousarray(np.asarray(Wr, dtype=np.float32))
    Wh = np.ascontiguousarray(np.asarray(Wh, dtype=np.float32))
    in_maps = [
        {
            "inputs": inputs[i * BC:(i + 1) * BC],
            "h_prev": h_prev[i * BC:(i + 1) * BC],
            "Wz": Wz,
            "Wr": Wr,
            "Wh": Wh,
        }
        for i in range(N_CORES)
    ]
    res = run_bass_kernel_spmd(
        nc, in_maps, core_ids=list(range(N_CORES)), trace=trace
    )
    out = np.concatenate([res.results[i]["out"] for i in range(N_CORES)], axis=0)
    return out, res


def kernel(inputs, h_prev, Wz, Wr, Wh):
    out, _ = run_sharded(inputs, h_prev, Wz, Wr, Wh, trace=False)
    return out



# revision 4
# speedup vs baseline: 1.0231x; 1.0231x over previous
"""Trainium2 Bass kernel for a custom GRU cell.

    x_h   = concat([inputs, h_prev], -1)            # [B, D+U]
    z     = sigmoid(x_h @ Wz)                       # [B, U]
    r     = sigmoid(x_h @ Wr)                       # [B, U]
    h_hat = tanh(concat([inputs, r * h_prev]) @ Wh) # [B, U]
    out   = z * h_prev + (1 - z) * h_hat

Data-parallel over 8 NeuronCores: batch sharded, weights replicated.

Per-core (B_c = 2048 rows, processed as 4 blocks of 512):
  - z and h matmuls in f32r (fp32 HIGH mode, 1 col/cycle)
  - r matmuls in fp8(e4m3) DoubleRow perf mode (2 k-slabs per pass,
    2x f32r throughput). Wr is pre-quantized on host to fp8 at scale
    32 and shipped as Wr8 [128, 8, 512]; the sigmoid reads psum with
    scale 1/32. xh8 fp8 staging tiles are produced by DVE (k<4) and
    GpSimd (k>=4) copies alongside the f32r xh tiles.
  - x_h transposed on the PE (f32r transpose) into feature-major
    k-tiles xh[k] [128, 512-batch], staged via PSUM, ACT copy to SBUF
  - r is computed TRANSPOSED (psum[u,b] = Wr8.T @ xh8) so r*h_prev
    feeds gate h's k>=4 lhsT directly with no extra transposes
  - combine is out = hh + z*(h - hh): 3 DVE ops, no ACT precompute
  - block-level software pipeline: gate-h of block i runs after z/r
    of block i+1
  - inputs/h_prev DMA'd one batched [128, 4, 512] transfer per block
    (block 0: per-tile transfers for lower first-use latency)
"""

import sys

for _p in ("/opt/trn_rl_repo", "/root/.axon_site/_ro/trn_rl_repo"):
    if _p not in sys.path:
        sys.path.append(_p)

import numpy as np
import ml_dtypes

FP8NP = ml_dtypes.float8_e4m3
WSCALE = 32.0

B, D, U = 16384, 512, 512
K = D + U
N_CORES = 8
BC = B // N_CORES          # rows per core (2048)
BB = 512                   # batch-block rows
NB = BC // BB              # blocks per core (4)
KC = K // 128              # contraction chunks (8)


def build_gru_tile_kernel(tc, d_in, d_hp, d_wz, d_wr8, d_wh, d_out, nb=NB):
    """Emit the GRU cell body into TileContext `tc`."""
    import contextlib

    from concourse import mybir
    from concourse.masks import make_identity

    f32 = mybir.dt.float32
    f32r = mybir.dt.float32r
    fp8 = mybir.dt.float8e4
    DR = mybir.MatmulPerfMode.DoubleRow
    nc = tc.nc
    Sig = mybir.ActivationFunctionType.Sigmoid
    Tanh = mybir.ActivationFunctionType.Tanh

    est = contextlib.ExitStack()
    sing = est.enter_context(tc.tile_pool(name="sing", bufs=1))
    wpool = est.enter_context(tc.tile_pool(name="w", bufs=1))
    io = est.enter_context(tc.tile_pool(name="io", bufs=2))
    io0 = est.enter_context(tc.tile_pool(name="io0", bufs=4))
    xhp = est.enter_context(tc.tile_pool(name="xhp", bufs=16))
    x8p = est.enter_context(tc.tile_pool(name="x8p", bufs=2))
    rhp = est.enter_context(tc.tile_pool(name="rhp", bufs=6))
    zp = est.enter_context(tc.tile_pool(name="zp", bufs=8))
    actp = est.enter_context(tc.tile_pool(name="act", bufs=4))
    tmpp = est.enter_context(tc.tile_pool(name="tmp", bufs=2))
    # PSUM: 3 transpose staging banks + 5 gate banks = 8/8
    pst = est.enter_context(tc.tile_pool(name="pst", bufs=3, space="PSUM"))
    psg = est.enter_context(tc.tile_pool(name="psg", bufs=5, space="PSUM"))

    ident0 = sing.tile([128, 128], f32)
    make_identity(nc, ident0)
    identr = sing.tile([128, 128], f32r)
    nc.scalar.copy(identr[:], ident0[:])

    # batched dram views: [bb][p, j, c] = t[bb*512 + j*128 + p, c]
    d_in4 = d_in.rearrange("(b j p) c -> b p j c", b=nb, j=4, p=128)
    d_hp4 = d_hp.rearrange("(b j p) c -> b p j c", b=nb, j=4, p=128)
    # weight views: [g][p, k, u] = W[(4g+k)*128 + p, u]
    d_wz2 = d_wz.rearrange("(g k p) u -> g p k u", g=2, k=4, p=128)
    d_wh2 = d_wh.rearrange("(g k p) u -> g p k u", g=2, k=4, p=128)

    # ---- DMA schedule ----
    # Block 0 x/h land per-tile (first transpose only waits on one
    # 256KB transfer); later blocks use one batched DMA each. Weights
    # stream in 1MB halves so gate-z's k=0 matmul isn't gated on the
    # full 2MB.
    pre_x = {}

    def load_x(bb):
        if bb == 0:
            xin, hps = [], []
            for j in range(4):
                r0 = 128 * j
                x_j = io0.tile([128, 512], f32r, tag="xin", name=f"x0_{j}")
                nc.sync.dma_start(x_j[:], d_in[r0:r0 + 128, :].bitcast(f32r))
                xin.append(x_j[:])
            for j in range(4):
                r0 = 128 * j
                h_j = io0.tile([128, 512], f32r, tag="hp", name=f"h0_{j}")
                nc.sync.dma_start(h_j[:], d_hp[r0:r0 + 128, :].bitcast(f32r))
                hps.append(h_j[:])
        else:
            xt = io.tile([128, 4, 512], f32r, tag="xin", name=f"x_{bb}")
            nc.sync.dma_start(xt[:], d_in4[bb].bitcast(f32r))
            ht = io.tile([128, 4, 512], f32r, tag="hp", name=f"h_{bb}")
            nc.sync.dma_start(ht[:], d_hp4[bb].bitcast(f32r))
            xin = [xt[:, j, :] for j in range(4)]
            hps = [ht[:, j, :] for j in range(4)]
        pre_x[bb] = (xin, hps)

    w_sb = {}

    def load_w2(name, dram2):
        t = wpool.tile([128, KC, 512], f32r, tag=name, name=name)
        for g in range(2):
            nc.sync.dma_start(t[:, 4 * g:4 * (g + 1), :], dram2[g].bitcast(f32r))
        w_sb[name] = t

    load_x(0)
    load_w2("wz", d_wz2)
    wr8 = wpool.tile([128, KC, 512], fp8, tag="wr8", name="wr8")
    nc.sync.dma_start(wr8[:], d_wr8)
    load_x(1)
    load_w2("wh", d_wh2)
    for bb in range(2, nb):
        load_x(bb)

    state = [None] * nb

    def phase_zr(bb):
        xin, hps = pre_x[bb]

        # ---- PE-transpose into feature-major k-tiles xh[k] [128, 512b],
        # with parallel fp8 copies into xh8 [128, 8, 512] ----
        xh = [None] * KC
        x8t = x8p.tile([128, KC, 512], fp8, tag="xh8", name=f"xh8_{bb}")

        def transpose_group(k):
            ps1 = pst.tile([128, 512], f32r, tag="pst", name=f"pst_{bb}_{k}")
            src = xin if k < 4 else hps
            kk = k % 4
            for j in range(4):
                nc.tensor.transpose(ps1[:, 128 * j:128 * (j + 1)],
                                    src[j][:, 128 * kk:128 * (kk + 1)], identr[:])
            sb1 = xhp.tile([128, 512], f32r, tag="xh", name=f"xh_{bb}_{k}")
            nc.scalar.copy(sb1[:], ps1[:])
            xh[k] = sb1[:]
            # fp8 staging for the r gate (DVE for x chunks, GpSimd for h)
            if k < 4:
                nc.vector.tensor_copy(x8t[:, k, :], sb1[:].bitcast(f32))
            else:
                nc.gpsimd.tensor_copy(x8t[:, k, :], sb1[:].bitcast(f32))

        for k in range(KC):
            transpose_group(k)

        # ---- gate z, batch-major f32r: ps[b,u] += xh[k][:,j].T @ Wz[k] ----
        zs = []
        for j in range(4):
            ps = psg.tile([128, 512], f32, tag="psg", name=f"psz_{bb}_{j}")
            for k in range(KC):
                nc.tensor.matmul(ps[:], xh[k][:, 128 * j:128 * (j + 1)],
                                 w_sb["wz"][:, k, :],
                                 start=(k == 0), stop=(k == KC - 1))
            z_j = zp.tile([128, 512], f32, tag="z", name=f"z_{bb}_{j}")
            nc.scalar.activation(z_j[:], ps[:], Sig)
            zs.append(z_j)

        # ---- gate r, transposed fp8 DoubleRow:
        #      ps[u,b] += Wr8[:,2k2:2k2+2,u].T @ xh8[:,2k2:2k2+2,:] ----
        rhT = []
        for u in range(4):
            ps = psg.tile([128, 512], f32, tag="psg", name=f"psr_{bb}_{u}")
            for k2 in range(4):
                nc.tensor.matmul(ps[:],
                                 wr8[:, 2 * k2:2 * k2 + 2, 128 * u:128 * (u + 1)],
                                 x8t[:, 2 * k2:2 * k2 + 2, :],
                                 start=(k2 == 0), stop=(k2 == 3),
                                 perf_mode=DR)
            rT_u = actp.tile([128, 512], f32, tag="rT", name=f"rT_{bb}_{u}")
            nc.scalar.activation(rT_u[:], ps[:], Sig, scale=1.0 / WSCALE)
            # rhT[u] = rT[u] * h_prev.T[u]  (hT = xh[4+u]), f32r out
            rh_u = rhp.tile([128, 512], f32r, tag="rhT", name=f"rh_{bb}_{u}")
            nc.vector.tensor_mul(rh_u[:], rT_u[:], xh[4 + u].bitcast(f32))
            rhT.append(rh_u)

        state[bb] = (xh, hps, zs, rhT)

    def phase_h(bb):
        xh, hps, zs, rhT = state[bb]
        for j in range(4):
            ps = psg.tile([128, 512], f32, tag="psg", name=f"psh_{bb}_{j}")
            for k in range(KC):
                lhs = (xh[k][:, 128 * j:128 * (j + 1)] if k < 4
                       else rhT[k - 4][:, 128 * j:128 * (j + 1)])
                nc.tensor.matmul(ps[:], lhs, w_sb["wh"][:, k, :],
                                 start=(k == 0), stop=(k == KC - 1))
            hh = actp.tile([128, 512], f32, tag="hh", name=f"hh_{bb}_{j}")
            nc.scalar.activation(hh[:], ps[:], Tanh)

            # out = hh + z * (h_prev - hh)
            z_j = zs[j]
            t = tmpp.tile([128, 512], f32, tag="tmp", name=f"t_{bb}_{j}")
            nc.vector.tensor_sub(t[:], hps[j].bitcast(f32), hh[:])
            t2 = tmpp.tile([128, 512], f32, tag="tmp2", name=f"t2_{bb}_{j}")
            nc.vector.tensor_mul(t2[:], z_j[:], t[:])
            out = tmpp.tile([128, 512], f32, tag="out", name=f"o_{bb}_{j}")
            nc.vector.tensor_add(out[:], hh[:], t2[:])
            r0 = bb * BB + 128 * j
            nc.sync.dma_start(d_out[r0:r0 + 128, :], out[:])
        state[bb] = None

    phase_zr(0)
    for bb in range(1, nb):
        phase_zr(bb)
        phase_h(bb - 1)
    phase_h(nb - 1)

    est.close()


_NC_CACHE = {}


def _build(nb=NB):
    if nb in _NC_CACHE:
        return _NC_CACHE[nb]
    import concourse.tile as tile
    from concourse import bacc, mybir

    f32 = mybir.dt.float32
    fp8 = mybir.dt.float8e4
    nc = bacc.Bacc("TRN2", target_bir_lowering=False, debug=False)
    d_in = nc.dram_tensor("inputs", [nb * BB, D], f32, kind="ExternalInput").ap()
    d_hp = nc.dram_tensor("h_prev", [nb * BB, U], f32, kind="ExternalInput").ap()
    d_wz = nc.dram_tensor("Wz", [K, U], f32, kind="ExternalInput").ap()
    d_wr8 = nc.dram_tensor("Wr8", [128, KC, 512], fp8, kind="ExternalInput").ap()
    d_wh = nc.dram_tensor("Wh", [K, U], f32, kind="ExternalInput").ap()
    d_out = nc.dram_tensor("out", [nb * BB, U], f32, kind="ExternalOutput").ap()

    with tile.TileContext(nc) as tc:
        build_gru_tile_kernel(tc, d_in, d_hp, d_wz, d_wr8, d_wh, d_out, nb=nb)
    nc.compile()
    _NC_CACHE[nb] = nc
    return nc


def run_sharded(inputs, h_prev, Wz, Wr, Wh, trace=False):
    from concourse.bass_utils import run_bass_kernel_spmd

    nc = _build()
    inputs = np.ascontiguousarray(np.asarray(inputs, dtype=np.float32))
    h_prev = np.ascontiguousarray(np.asarray(h_prev, dtype=np.float32))
    Wz = np.ascontiguousarray(np.asarray(Wz, dtype=np.float32))
    Wh = np.ascontiguousarray(np.asarray(Wh, dtype=np.float32))
    # host-side fp8 weight prep for the r gate: [p, k, u] = q8(32*Wr[128k+p, u])
    Wr8 = np.ascontiguousarray(
        (WSCALE * np.asarray(Wr, dtype=np.float32))
        .reshape(KC, 128, U).transpose(1, 0, 2).astype(FP8NP)
    )
    in_maps = [
        {
            "inputs": inputs[i * BC:(i + 1) * BC],
            "h_prev": h_prev[i * BC:(i + 1) * BC],
            "Wz": Wz,
            "Wr8": Wr8,
            "Wh": Wh,
        }
        for i in range(N_CORES)
    ]
    res = run_bass_kernel_spmd(
        nc, in_maps, core_ids=list(range(N_CORES)), trace=trace
    )
    out = np.concatenate([res.results[i]["out"] for i in range(N_CORES)], axis=0)
    return out, res


def kernel(inputs, h_prev, Wz, Wr, Wh):
    out, _ = run_sharded(inputs, h_prev, Wz, Wr, Wh, trace=False)
    return out


# revision 8
# speedup vs baseline: 1.0284x; 1.0052x over previous
"""Trainium2 Bass kernel for a custom GRU cell.

    x_h   = concat([inputs, h_prev], -1)            # [B, D+U]
    z     = sigmoid(x_h @ Wz)                       # [B, U]
    r     = sigmoid(x_h @ Wr)                       # [B, U]
    h_hat = tanh(concat([inputs, r * h_prev]) @ Wh) # [B, U]
    out   = z * h_prev + (1 - z) * h_hat

Data-parallel over 8 NeuronCores: batch sharded, weights replicated.

Per-core (B_c = 2048 rows, processed as 4 blocks of 512):
  - z and h matmuls in f32r (fp32 HIGH mode, 1 col/cycle)
  - r matmuls in fp8(e4m3) DoubleRow perf mode (2 k-slabs per pass,
    2x f32r throughput). Wr is pre-quantized on host to fp8 at scale
    32 and shipped as Wr8 [128, 8, 512]; the sigmoid reads psum with
    scale 1/32. xh8 fp8 staging tiles are produced by DVE (k<4) and
    GpSimd (k>=4) copies alongside the f32r xh tiles.
  - x_h transposed on the PE (f32r transpose) into feature-major
    k-tiles xh[k] [128, 512-batch], staged via PSUM, ACT copy to SBUF
  - r is computed TRANSPOSED (psum[u,b] = Wr8.T @ xh8) so r*h_prev
    feeds gate h's k>=4 lhsT directly with no extra transposes
  - combine is out = hh + z*(h - hh): 3 DVE ops, no ACT precompute
  - block-level software pipeline: gate-h of block i runs after z/r
    of block i+1
  - inputs/h_prev DMA'd one batched [128, 4, 512] transfer per block
    (block 0: per-tile transfers for lower first-use latency)
"""

import sys

for _p in ("/opt/trn_rl_repo", "/root/.axon_site/_ro/trn_rl_repo"):
    if _p not in sys.path:
        sys.path.append(_p)

import numpy as np
import ml_dtypes

FP8NP = ml_dtypes.float8_e4m3
WSCALE = 32.0

B, D, U = 16384, 512, 512
K = D + U
N_CORES = 8
BC = B // N_CORES          # rows per core (2048)
BB = 512                   # batch-block rows
NB = BC // BB              # blocks per core (4)
KC = K // 128              # contraction chunks (8)


def build_gru_tile_kernel(tc, d_in, d_hp, d_wz, d_wr8, d_wh, d_out, nb=NB):
    """Emit the GRU cell body into TileContext `tc`."""
    import contextlib

    from concourse import mybir
    from concourse.masks import make_identity

    f32 = mybir.dt.float32
    f32r = mybir.dt.float32r
    fp8 = mybir.dt.float8e4
    DR = mybir.MatmulPerfMode.DoubleRow
    nc = tc.nc
    Sig = mybir.ActivationFunctionType.Sigmoid
    Tanh = mybir.ActivationFunctionType.Tanh

    est = contextlib.ExitStack()
    sing = est.enter_context(tc.tile_pool(name="sing", bufs=1))
    wpool = est.enter_context(tc.tile_pool(name="w", bufs=1))
    io = est.enter_context(tc.tile_pool(name="io", bufs=2))
    io0 = est.enter_context(tc.tile_pool(name="io0", bufs=4))
    xhp = est.enter_context(tc.tile_pool(name="xhp", bufs=16))
    x8p = est.enter_context(tc.tile_pool(name="x8p", bufs=2))
    rhp = est.enter_context(tc.tile_pool(name="rhp", bufs=6))
    zp = est.enter_context(tc.tile_pool(name="zp", bufs=8))
    zxp = est.enter_context(tc.tile_pool(name="zxp", bufs=4))
    actp = est.enter_context(tc.tile_pool(name="act", bufs=4))
    tmpp = est.enter_context(tc.tile_pool(name="tmp", bufs=2))
    # PSUM: 3 transpose staging banks + 5 gate banks = 8/8
    pst = est.enter_context(tc.tile_pool(name="pst", bufs=3, space="PSUM"))
    psg = est.enter_context(tc.tile_pool(name="psg", bufs=5, space="PSUM"))

    ident0 = sing.tile([128, 128], f32)
    make_identity(nc, ident0)
    identr = sing.tile([128, 128], f32r)
    nc.scalar.copy(identr[:], ident0[:])

    # batched dram views: [bb][p, j, c] = t[bb*512 + j*128 + p, c]
    d_in4 = d_in.rearrange("(b j p) c -> b p j c", b=nb, j=4, p=128)
    d_hp4 = d_hp.rearrange("(b j p) c -> b p j c", b=nb, j=4, p=128)
    # weight views: [g][p, k, u] = W[(4g+k)*128 + p, u]
    d_wz2 = d_wz.rearrange("(g k p) u -> g p k u", g=2, k=4, p=128)
    d_wh2 = d_wh.rearrange("(g k p) u -> g p k u", g=2, k=4, p=128)

    # ---- DMA schedule ----
    # Block 0 x/h land per-tile (first transpose only waits on one
    # 256KB transfer); later blocks use one batched DMA each. Weights
    # stream in 1MB halves so gate-z's k=0 matmul isn't gated on the
    # full 2MB.
    pre_x = {}

    def load_x(bb):
        if bb == 0:
            xin, hps = [], []
            for j in range(4):
                r0 = 128 * j
                x_j = io0.tile([128, 512], f32r, tag="xin", name=f"x0_{j}")
                nc.sync.dma_start(x_j[:], d_in[r0:r0 + 128, :].bitcast(f32r))
                xin.append(x_j[:])
            for j in range(4):
                r0 = 128 * j
                h_j = io0.tile([128, 512], f32r, tag="hp", name=f"h0_{j}")
                nc.sync.dma_start(h_j[:], d_hp[r0:r0 + 128, :].bitcast(f32r))
                hps.append(h_j[:])
        else:
            xt = io.tile([128, 4, 512], f32r, tag="xin", name=f"x_{bb}")
            nc.sync.dma_start(xt[:], d_in4[bb].bitcast(f32r))
            ht = io.tile([128, 4, 512], f32r, tag="hp", name=f"h_{bb}")
            nc.sync.dma_start(ht[:], d_hp4[bb].bitcast(f32r))
            xin = [xt[:, j, :] for j in range(4)]
            hps = [ht[:, j, :] for j in range(4)]
        pre_x[bb] = (xin, hps)

    w_sb = {}

    def load_w2(name, dram2):
        t = wpool.tile([128, KC, 512], f32r, tag=name, name=name)
        for g in range(2):
            nc.sync.dma_start(t[:, 4 * g:4 * (g + 1), :], dram2[g].bitcast(f32r))
        w_sb[name] = t

    load_x(0)
    load_w2("wz", d_wz2)
    wr8 = wpool.tile([128, KC, 512], fp8, tag="wr8", name="wr8")
    nc.sync.dma_start(wr8[:], d_wr8)
    load_x(1)
    load_w2("wh", d_wh2)
    for bb in range(2, nb):
        load_x(bb)

    state = [None] * nb

    def phase_zr(bb):
        xin, hps = pre_x[bb]

        # ---- PE-transpose into feature-major k-tiles xh[k] [128, 512b],
        # with parallel fp8 copies into xh8 [128, 8, 512] ----
        xh = [None] * KC
        x8t = x8p.tile([128, KC, 512], fp8, tag="xh8", name=f"xh8_{bb}")

        def transpose_group(k):
            ps1 = pst.tile([128, 512], f32r, tag="pst", name=f"pst_{bb}_{k}")
            src = xin if k < 4 else hps
            kk = k % 4
            for j in range(4):
                nc.tensor.transpose(ps1[:, 128 * j:128 * (j + 1)],
                                    src[j][:, 128 * kk:128 * (kk + 1)], identr[:])
            sb1 = xhp.tile([128, 512], f32r, tag="xh", name=f"xh_{bb}_{k}")
            nc.scalar.copy(sb1[:], ps1[:])
            xh[k] = sb1[:]
            # fp8 staging for the r gate (DVE 8-bit-out casts are ~423ns)
            nc.vector.tensor_copy(x8t[:, k, :], sb1[:].bitcast(f32))

        for k in range(KC):
            transpose_group(k)

        # ---- gate z, batch-major f32r: ps[b,u] += xh[k][:,j].T @ Wz[k] ----
        zs = []
        for j in range(4):
            ps = psg.tile([128, 512], f32, tag="psg", name=f"psz_{bb}_{j}")
            for k in range(KC):
                nc.tensor.matmul(ps[:], xh[k][:, 128 * j:128 * (j + 1)],
                                 w_sb["wz"][:, k, :],
                                 start=(k == 0), stop=(k == KC - 1))
            z_j = zp.tile([128, 512], f32, tag="z", name=f"z_{bb}_{j}")
            nc.scalar.activation(z_j[:], ps[:], Sig)
            if bb == nb - 1:
                # tail block: precompute zc = 1-z (ACT) and zh = z*h (DVE)
                # so the post-tanh chain is only mul+add
                zc_j = zxp.tile([128, 512], f32, tag="zc", name=f"zc_{bb}_{j}")
                nc.scalar.activation(zc_j[:], z_j[:],
                                     mybir.ActivationFunctionType.Copy,
                                     bias=1.0, scale=-1.0)
                zh_j = zxp.tile([128, 512], f32, tag="zh", name=f"zh_{bb}_{j}")
                nc.vector.tensor_mul(zh_j[:], z_j[:], hps[j].bitcast(f32))
                zs.append((zc_j, zh_j))
            else:
                zs.append(z_j)

        # ---- gate r, transposed fp8 DoubleRow:
        #      ps[u,b] += Wr8[:,2k2:2k2+2,u].T @ xh8[:,2k2:2k2+2,:] ----
        rhT = []
        for u in range(4):
            ps = psg.tile([128, 512], f32, tag="psg", name=f"psr_{bb}_{u}")
            for k2 in range(4):
                nc.tensor.matmul(ps[:],
                                 wr8[:, 2 * k2:2 * k2 + 2, 128 * u:128 * (u + 1)],
                                 x8t[:, 2 * k2:2 * k2 + 2, :],
                                 start=(k2 == 0), stop=(k2 == 3),
                                 perf_mode=DR)
            rT_u = actp.tile([128, 512], f32, tag="rT", name=f"rT_{bb}_{u}")
            nc.scalar.activation(rT_u[:], ps[:], Sig, scale=1.0 / WSCALE)
            # rhT[u] = rT[u] * h_prev.T[u]  (hT = xh[4+u]), f32r out
            rh_u = rhp.tile([128, 512], f32r, tag="rhT", name=f"rh_{bb}_{u}")
            nc.vector.tensor_mul(rh_u[:], rT_u[:], xh[4 + u].bitcast(f32))
            rhT.append(rh_u)

        state[bb] = (xh, hps, zs, rhT)

    def phase_h(bb):
        xh, hps, zs, rhT = state[bb]
        for j in range(4):
            ps = psg.tile([128, 512], f32, tag="psg", name=f"psh_{bb}_{j}")
            for k in range(KC):
                lhs = (xh[k][:, 128 * j:128 * (j + 1)] if k < 4
                       else rhT[k - 4][:, 128 * j:128 * (j + 1)])
                nc.tensor.matmul(ps[:], lhs, w_sb["wh"][:, k, :],
                                 start=(k == 0), stop=(k == KC - 1))
            r0 = bb * BB + 128 * j
            if bb == nb - 1:
                # tail block: out = zc*hh + zh, split into halves so the
                # final activation+combine+DMA chain drains in 256-col
                # pieces instead of one 512-col chain
                zc_j, zh_j = zs[j]
                halves = 2 if j == 3 else 1
                w = 512 // halves
                for s in range(halves):
                    sl = slice(w * s, w * (s + 1))
                    hh = actp.tile([128, 512], f32, tag="hh",
                                   name=f"hh_{bb}_{j}_{s}")
                    nc.scalar.activation(hh[:, sl], ps[:, sl], Tanh)
                    t2 = tmpp.tile([128, 512], f32, tag="tmp2",
                                   name=f"t2_{bb}_{j}_{s}")
                    nc.vector.tensor_mul(t2[:, sl], zc_j[:, sl], hh[:, sl])
                    out = tmpp.tile([128, 512], f32, tag="out",
                                    name=f"o_{bb}_{j}_{s}")
                    nc.vector.tensor_add(out[:, sl], t2[:, sl], zh_j[:, sl])
                    nc.sync.dma_start(d_out[r0:r0 + 128, sl], out[:, sl])
            else:
                hh = actp.tile([128, 512], f32, tag="hh", name=f"hh_{bb}_{j}")
                nc.scalar.activation(hh[:], ps[:], Tanh)
                # out = hh + z * (h_prev - hh)
                z_j = zs[j]
                t = tmpp.tile([128, 512], f32, tag="tmp", name=f"t_{bb}_{j}")
                nc.vector.tensor_sub(t[:], hps[j].bitcast(f32), hh[:])
                t2 = tmpp.tile([128, 512], f32, tag="tmp2", name=f"t2_{bb}_{j}")
                nc.vector.tensor_mul(t2[:], z_j[:], t[:])
                out = tmpp.tile([128, 512], f32, tag="out", name=f"o_{bb}_{j}")
                nc.vector.tensor_add(out[:], hh[:], t2[:])
                nc.sync.dma_start(d_out[r0:r0 + 128, :], out[:])
        state[bb] = None

    phase_zr(0)
    for bb in range(1, nb):
        phase_zr(bb)
        phase_h(bb - 1)
    phase_h(nb - 1)

    est.close()


_NC_CACHE = {}


def _build(nb=NB):
    if nb in _NC_CACHE:
        return _NC_CACHE[nb]
    import concourse.tile as tile
    from concourse import bacc, mybir

    f32 = mybir.dt.float32
    fp8 = mybir.dt.float8e4
    nc = bacc.Bacc("TRN2", target_bir_lowering=False, debug=False)
    d_in = nc.dram_tensor("inputs", [nb * BB, D], f32, kind="ExternalInput").ap()
    d_hp = nc.dram_tensor("h_prev", [nb * BB, U], f32, kind="ExternalInput").ap()
    d_wz = nc.dram_tensor("Wz", [K, U], f32, kind="ExternalInput").ap()
    d_wr8 = nc.dram_tensor("Wr8", [128, KC, 512], fp8, kind="ExternalInput").ap()
    d_wh = nc.dram_tensor("Wh", [K, U], f32, kind="ExternalInput").ap()
    d_out = nc.dram_tensor("out", [nb * BB, U], f32, kind="ExternalOutput").ap()

    with tile.TileContext(nc) as tc:
        build_gru_tile_kernel(tc, d_in, d_hp, d_wz, d_wr8, d_wh, d_out, nb=nb)
    nc.compile()
    _NC_CACHE[nb] = nc
    return nc


def run_sharded(inputs, h_prev, Wz, Wr, Wh, trace=False):
    from concourse.bass_utils import run_bass_kernel_spmd

    nc = _build()
    inputs = np.ascontiguousarray(np.asarray(inputs, dtype=np.float32))
    h_prev = np.ascontiguousarray(np.asarray(h_prev, dtype=np.float32))
    Wz = np.ascontiguousarray(np.asarray(Wz, dtype=np.float32))
    Wh = np.ascontiguousarray(np.asarray(Wh, dtype=np.float32))
    # host-side fp8 weight prep for the r gate: [p, k, u] = q8(32*Wr[128k+p, u])
    Wr8 = np.ascontiguousarray(
        (WSCALE * np.asarray(Wr, dtype=np.float32))
        .reshape(KC, 128, U).transpose(1, 0, 2).astype(FP8NP)
    )
    in_maps = [
        {
            "inputs": inputs[i * BC:(i + 1) * BC],
            "h_prev": h_prev[i * BC:(i + 1) * BC],
            "Wz": Wz,
            "Wr8": Wr8,
            "Wh": Wh,
        }
        for i in range(N_CORES)
    ]
    res = run_bass_kernel_spmd(
        nc, in_maps, core_ids=list(range(N_CORES)), trace=trace
    )
    out = np.concatenate([res.results[i]["out"] for i in range(N_CORES)], axis=0)
    return out, res


def kernel(inputs, h_prev, Wz, Wr, Wh):
    out, _ = run_sharded(inputs, h_prev, Wz, Wr, Wh, trace=False)
    return out


# revision 10
# speedup vs baseline: 1.1045x; 1.0741x over previous
"""Trainium2 Bass kernel for a custom GRU cell.

    x_h   = concat([inputs, h_prev], -1)            # [B, D+U]
    z     = sigmoid(x_h @ Wz)                       # [B, U]
    r     = sigmoid(x_h @ Wr)                       # [B, U]
    h_hat = tanh(concat([inputs, r * h_prev]) @ Wh) # [B, U]
    out   = z * h_prev + (1 - z) * h_hat

Data-parallel over 8 NeuronCores: batch sharded, weights replicated.

Per-core (B_c = 2048 rows, processed as 4 blocks of 512):
  - z and h matmuls in f32r (fp32 HIGH mode, 1 col/cycle)
  - r matmuls in fp8(e4m3) DoubleRow perf mode (2 k-slabs per pass,
    2x f32r throughput). Wr is pre-quantized on host to fp8 at scale
    32 and shipped as Wr8 [128, 8, 512]; the sigmoid reads psum with
    scale 1/32. xh8 fp8 staging tiles are produced by DVE (k<4) and
    GpSimd (k>=4) copies alongside the f32r xh tiles.
  - x_h transposed on the PE (f32r transpose) into feature-major
    k-tiles xh[k] [128, 512-batch], staged via PSUM, ACT copy to SBUF
  - r is computed TRANSPOSED (psum[u,b] = Wr8.T @ xh8) so r*h_prev
    feeds gate h's k>=4 lhsT directly with no extra transposes
  - combine is out = hh + z*(h - hh): 3 DVE ops, no ACT precompute
  - block-level software pipeline: gate-h of block i runs after z/r
    of block i+1
  - inputs/h_prev DMA'd one batched [128, 4, 512] transfer per block
    (block 0: per-tile transfers for lower first-use latency)
"""

import sys

for _p in ("/opt/trn_rl_repo", "/root/.axon_site/_ro/trn_rl_repo"):
    if _p not in sys.path:
        sys.path.append(_p)

import numpy as np
import ml_dtypes

FP8NP = ml_dtypes.float8_e4m3
WSCALE = 32.0

B, D, U = 16384, 512, 512
K = D + U
N_CORES = 8
BC = B // N_CORES          # rows per core (2048)
BB = 512                   # batch-block rows
NB = BC // BB              # blocks per core (4)
KC = K // 128              # contraction chunks (8)


def build_gru_tile_kernel(tc, d_in, d_hp, d_wz, d_wr8, d_wh, d_out, nb=NB):
    """Emit the GRU cell body into TileContext `tc`."""
    import contextlib

    from concourse import mybir
    from concourse.masks import make_identity

    f32 = mybir.dt.float32
    f32r = mybir.dt.float32r
    fp8 = mybir.dt.float8e4
    DR = mybir.MatmulPerfMode.DoubleRow
    nc = tc.nc
    Sig = mybir.ActivationFunctionType.Sigmoid
    Tanh = mybir.ActivationFunctionType.Tanh

    est = contextlib.ExitStack()
    sing = est.enter_context(tc.tile_pool(name="sing", bufs=1))
    wpool = est.enter_context(tc.tile_pool(name="w", bufs=1))
    io = est.enter_context(tc.tile_pool(name="io", bufs=1))
    io0 = est.enter_context(tc.tile_pool(name="io0", bufs=8))
    xhp = est.enter_context(tc.tile_pool(name="xhp", bufs=16))
    x8p = est.enter_context(tc.tile_pool(name="x8p", bufs=2))
    rhp = est.enter_context(tc.tile_pool(name="rhp", bufs=6))
    zp = est.enter_context(tc.tile_pool(name="zp", bufs=8))
    zxp = est.enter_context(tc.tile_pool(name="zxp", bufs=4))
    actp = est.enter_context(tc.tile_pool(name="act", bufs=4))
    tmpp = est.enter_context(tc.tile_pool(name="tmp", bufs=2))
    outp = est.enter_context(tc.tile_pool(name="out", bufs=6))
    # PSUM: 3 transpose staging banks + 5 gate banks = 8/8
    pst = est.enter_context(tc.tile_pool(name="pst", bufs=3, space="PSUM"))
    psg = est.enter_context(tc.tile_pool(name="psg", bufs=5, space="PSUM"))

    ident0 = sing.tile([128, 128], f32)
    make_identity(nc, ident0)
    identr = sing.tile([128, 128], f32r)
    nc.scalar.copy(identr[:], ident0[:])

    # batched dram views: [bb][p, j, c] = t[bb*512 + j*128 + p, c]
    d_in4 = d_in.rearrange("(b j p) c -> b p j c", b=nb, j=4, p=128)
    d_hp4 = d_hp.rearrange("(b j p) c -> b p j c", b=nb, j=4, p=128)
    # weight views: [g][p, k, u] = W[(4g+k)*128 + p, u]
    d_wz2 = d_wz.rearrange("(g k p) u -> g p k u", g=2, k=4, p=128)
    d_wh2 = d_wh.rearrange("(g k p) u -> g p k u", g=2, k=4, p=128)

    # ---- DMA schedule ----
    # Block 0 x/h land per-tile (first transpose only waits on one
    # 256KB transfer); later blocks use one batched DMA each. Weights
    # stream in 1MB halves so gate-z's k=0 matmul isn't gated on the
    # full 2MB.
    pre_x = {}

    def load_x(bb):
        if bb != 1:
            xin, hps = [], []
            for j in range(4):
                r0 = bb * BB + 128 * j
                x_j = io0.tile([128, 512], f32r, tag="xin", name=f"x{bb}_{j}")
                nc.sync.dma_start(x_j[:], d_in[r0:r0 + 128, :].bitcast(f32r))
                xin.append(x_j[:])
            for j in range(4):
                r0 = bb * BB + 128 * j
                h_j = io0.tile([128, 512], f32r, tag="hp", name=f"h{bb}_{j}")
                nc.sync.dma_start(h_j[:], d_hp[r0:r0 + 128, :].bitcast(f32r))
                hps.append(h_j[:])
        else:
            xt = io.tile([128, 4, 512], f32r, tag="xin", name=f"x_{bb}")
            nc.sync.dma_start(xt[:], d_in4[bb].bitcast(f32r))
            ht = io.tile([128, 4, 512], f32r, tag="hp", name=f"h_{bb}")
            nc.sync.dma_start(ht[:], d_hp4[bb].bitcast(f32r))
            xin = [xt[:, j, :] for j in range(4)]
            hps = [ht[:, j, :] for j in range(4)]
        pre_x[bb] = (xin, hps)

    w_sb = {}

    def load_w2(name, dram2):
        t = wpool.tile([128, KC, 512], f32r, tag=name, name=name)
        for g in range(2):
            nc.sync.dma_start(t[:, 4 * g:4 * (g + 1), :], dram2[g].bitcast(f32r))
        w_sb[name] = t

    load_x(0)
    load_w2("wz", d_wz2)
    wr8 = wpool.tile([128, KC, 512], fp8, tag="wr8", name="wr8")
    nc.sync.dma_start(wr8[:], d_wr8)
    load_x(1)
    load_w2("wh", d_wh2)
    for bb in range(2, nb):
        load_x(bb)

    state = [None] * nb

    def phase_zr(bb):
        xin, hps = pre_x[bb]

        # ---- PE-transpose into feature-major k-tiles xh[k] [128, 512b],
        # with parallel fp8 copies into xh8 [128, 8, 512] ----
        xh = [None] * KC
        x8t = x8p.tile([128, KC, 512], fp8, tag="xh8", name=f"xh8_{bb}")

        def transpose_group(k):
            ps1 = pst.tile([128, 512], f32r, tag="pst", name=f"pst_{bb}_{k}")
            src = xin if k < 4 else hps
            kk = k % 4
            for j in range(4):
                nc.tensor.transpose(ps1[:, 128 * j:128 * (j + 1)],
                                    src[j][:, 128 * kk:128 * (kk + 1)], identr[:])
            sb1 = xhp.tile([128, 512], f32r, tag="xh", name=f"xh_{bb}_{k}")
            nc.scalar.copy(sb1[:], ps1[:])
            xh[k] = sb1[:]
            # fp8 staging for the r gate (DVE 8-bit-out casts are ~423ns)
            nc.vector.tensor_copy(x8t[:, k, :], sb1[:].bitcast(f32))

        for k in range(KC):
            transpose_group(k)

        # ---- gate z, batch-major f32r: ps[b,u] += xh[k][:,j].T @ Wz[k] ----
        zs = []
        for j in range(4):
            ps = psg.tile([128, 512], f32, tag="psg", name=f"psz_{bb}_{j}")
            for k in range(KC):
                nc.tensor.matmul(ps[:], xh[k][:, 128 * j:128 * (j + 1)],
                                 w_sb["wz"][:, k, :],
                                 start=(k == 0), stop=(k == KC - 1))
            z_j = zp.tile([128, 512], f32, tag="z", name=f"z_{bb}_{j}")
            nc.scalar.activation(z_j[:], ps[:], Sig)
            if bb == nb - 1:
                # tail block: precompute zc = 1-z (ACT) and zh = z*h (DVE)
                # so the post-tanh chain is only mul+add
                zc_j = zxp.tile([128, 512], f32, tag="zc", name=f"zc_{bb}_{j}")
                nc.scalar.activation(zc_j[:], z_j[:],
                                     mybir.ActivationFunctionType.Copy,
                                     bias=1.0, scale=-1.0)
                zh_j = zxp.tile([128, 512], f32, tag="zh", name=f"zh_{bb}_{j}")
                nc.vector.tensor_mul(zh_j[:], z_j[:], hps[j].bitcast(f32))
                zs.append((zc_j, zh_j))
            else:
                zs.append(z_j)

        # ---- gate r, transposed fp8 DoubleRow:
        #      ps[u,b] += Wr8[:,2k2:2k2+2,u].T @ xh8[:,2k2:2k2+2,:] ----
        rhT = []
        for u in range(4):
            ps = psg.tile([128, 512], f32, tag="psg", name=f"psr_{bb}_{u}")
            for k2 in range(4):
                nc.tensor.matmul(ps[:],
                                 wr8[:, 2 * k2:2 * k2 + 2, 128 * u:128 * (u + 1)],
                                 x8t[:, 2 * k2:2 * k2 + 2, :],
                                 start=(k2 == 0), stop=(k2 == 3),
                                 perf_mode=DR)
            rT_u = actp.tile([128, 512], f32, tag="rT", name=f"rT_{bb}_{u}")
            nc.scalar.activation(rT_u[:], ps[:], Sig, scale=1.0 / WSCALE)
            # rhT[u] = rT[u] * h_prev.T[u]  (hT = xh[4+u]), f32r out
            rh_u = rhp.tile([128, 512], f32r, tag="rhT", name=f"rh_{bb}_{u}")
            nc.vector.tensor_mul(rh_u[:], rT_u[:], xh[4 + u].bitcast(f32))
            rhT.append(rh_u)

        state[bb] = (xh, hps, zs, rhT)

    def phase_h(bb):
        xh, hps, zs, rhT = state[bb]
        for j in range(4):
            ps = psg.tile([128, 512], f32, tag="psg", name=f"psh_{bb}_{j}")
            for k in range(KC):
                lhs = (xh[k][:, 128 * j:128 * (j + 1)] if k < 4
                       else rhT[k - 4][:, 128 * j:128 * (j + 1)])
                nc.tensor.matmul(ps[:], lhs, w_sb["wh"][:, k, :],
                                 start=(k == 0), stop=(k == KC - 1))
            r0 = bb * BB + 128 * j
            if bb == nb - 1:
                # tail block: out = zc*hh + zh, split into halves so the
                # final activation+combine+DMA chain drains in 256-col
                # pieces instead of one 512-col chain
                zc_j, zh_j = zs[j]
                halves = 2 if j == 3 else 1
                w = 512 // halves
                for s in range(halves):
                    sl = slice(w * s, w * (s + 1))
                    hh = actp.tile([128, 512], f32, tag="hh",
                                   name=f"hh_{bb}_{j}_{s}")
                    nc.scalar.activation(hh[:, sl], ps[:, sl], Tanh)
                    t2 = tmpp.tile([128, 512], f32, tag="tmp2",
                                   name=f"t2_{bb}_{j}_{s}")
                    nc.vector.tensor_mul(t2[:, sl], zc_j[:, sl], hh[:, sl])
                    out = outp.tile([128, 512], f32, tag="out",
                                    name=f"o_{bb}_{j}_{s}")
                    nc.vector.tensor_add(out[:, sl], t2[:, sl], zh_j[:, sl])
                    nc.sync.dma_start(d_out[r0:r0 + 128, sl], out[:, sl])
            else:
                hh = actp.tile([128, 512], f32, tag="hh", name=f"hh_{bb}_{j}")
                nc.scalar.activation(hh[:], ps[:], Tanh)
                # out = hh + z * (h_prev - hh)
                z_j = zs[j]
                t = tmpp.tile([128, 512], f32, tag="tmp", name=f"t_{bb}_{j}")
                nc.vector.tensor_sub(t[:], hps[j].bitcast(f32), hh[:])
                t2 = tmpp.tile([128, 512], f32, tag="tmp2", name=f"t2_{bb}_{j}")
                nc.vector.tensor_mul(t2[:], z_j[:], t[:])
                out = outp.tile([128, 512], f32, tag="out", name=f"o_{bb}_{j}")
                nc.vector.tensor_add(out[:], hh[:], t2[:])
                nc.sync.dma_start(d_out[r0:r0 + 128, :], out[:])
        state[bb] = None

    phase_zr(0)
    for bb in range(1, nb):
        phase_zr(bb)
        phase_h(bb - 1)
    phase_h(nb - 1)

    est.close()


_NC_CACHE = {}


def _build(nb=NB):
    if nb in _NC_CACHE:
        return _NC_CACHE[nb]
    import concourse.tile as tile
    from concourse import bacc, mybir

    f32 = mybir.dt.float32
    fp8 = mybir.dt.float8e4
    nc = bacc.Bacc("TRN2", target_bir_lowering=False, debug=False)
    d_in = nc.dram_tensor("inputs", [nb * BB, D], f32, kind="ExternalInput").ap()
    d_hp = nc.dram_tensor("h_prev", [nb * BB, U], f32, kind="ExternalInput").ap()
    d_wz = nc.dram_tensor("Wz", [K, U], f32, kind="ExternalInput").ap()
    d_wr8 = nc.dram_tensor("Wr8", [128, KC, 512], fp8, kind="ExternalInput").ap()
    d_wh = nc.dram_tensor("Wh", [K, U], f32, kind="ExternalInput").ap()
    d_out = nc.dram_tensor("out", [nb * BB, U], f32, kind="ExternalOutput").ap()

    with tile.TileContext(nc) as tc:
        build_gru_tile_kernel(tc, d_in, d_hp, d_wz, d_wr8, d_wh, d_out, nb=nb)
    nc.compile()
    _NC_CACHE[nb] = nc
    return nc


def run_sharded(inputs, h_prev, Wz, Wr, Wh, trace=False):
    from concourse.bass_utils import run_bass_kernel_spmd

    nc = _build()
    inputs = np.ascontiguousarray(np.asarray(inputs, dtype=np.float32))
    h_prev = np.ascontiguousarray(np.asarray(h_prev, dtype=np.float32))
    Wz = np.ascontiguousarray(np.asarray(Wz, dtype=np.float32))
    Wh = np.ascontiguousarray(np.asarray(Wh, dtype=np.float32))
    # host-side fp8 weight prep for the r gate: [p, k, u] = q8(32*Wr[128k+p, u])
    Wr8 = np.ascontiguousarray(
        (WSCALE * np.asarray(Wr, dtype=np.float32))
        .reshape(KC, 128, U).transpose(1, 0, 2).astype(FP8NP)
    )
    in_maps = [
        {
            "inputs": inputs[i * BC:(i + 1) * BC],
            "h_prev": h_prev[i * BC:(i + 1) * BC],
            "Wz": Wz,
            "Wr8": Wr8,
            "Wh": Wh,
        }
        for i in range(N_CORES)
    ]
    res = run_bass_kernel_spmd(
        nc, in_maps, core_ids=list(range(N_CORES)), trace=trace
    )
    out = np.concatenate([res.results[i]["out"] for i in range(N_CORES)], axis=0)
    return out, res


def kernel(inputs, h_prev, Wz, Wr, Wh):
    out, _ = run_sharded(inputs, h_prev, Wz, Wr, Wh, trace=False)
    return out
